# revision 19
# baseline (speedup 1.0000x reference)
"""CLUBMean loss kernel for Trainium2, 8-core data-parallel.

Math: with x_vec = mean_s(x), y_vec = mean_s(y), mu = MLP(x_vec):
  positive_i = -||mu_i - y_i||^2 / 2
  negative_i = -(S2/N - 2 mu_i . Ey + ||mu_i||^2) / 2      (exact expansion)
  loss = mean_i(positive_i - negative_i)

Each core handles 128 of the 1024 samples and emits a stat tile [128, 22]
of partial sums (see COLS below); the host combines in float64.

Pipeline per core (v2 -- tail-optimized):
  - one HWDGE (sync) queue streams 16 x-chunks then 8 y-chunks (1 MiB,
    32 channels x 64 spatial); first chunk split in half, last y chunk
    split in quarters so the trailing DVE reduce is short
  - spatial pooling: DVE tensor_reduce; GPSIMD folds 64->32 for alternate
    chunks (FOLD) to keep DVE ahead of the stream
  - x path: PE transposes pooled vectors to channel-major, MLP as fp32
    PSUM matmuls (accumulation groups contiguous), ReLU/bias on ACT
  - as soon as mu is ready (~x-stream end) PE transposes it back to
    sample-major; ACT copies with scale=64 -> mu64; B/Mu stats from muT
  - y chunks never touch PE transposes: per chunk, DVE computes
    dtmp = xv_y - mu64 (xv_y holds 64*y_vec), ACT Square+accum_out gives
    per-sample A and S2 partials, PE does a tiny ones-matmul for Ey
  - stat cols untouched by the last chunk ship early (ACT-issued DMA
    while the stream still runs); the last 3 cols ship right after the
    final quarter lands -> tail after last byte is ~3 us

Each DMA's +16 semaphore arrives as +1 per DGE lane, so chunk completion
uses one semaphore per transfer (cumulative thresholds are unsound).

Host combine (f64): A,S2 carry 64^2 scale, Ey carries 64; exact /4096, /64.
  cols 0-6  : A partials  (y chunks 16-22), per sample, x4096
  cols 7-13 : S2 partials (y chunks 16-22), per sample, x4096
  cols 14,15: B halves, per channel
  cols 16,17: Mu halves, per channel
  col  18   : Ey half0, per channel, x64
  col  19   : A chunk 23 | col 20: S2 chunk 23 | col 21: Ey half1 x64
"""

import sys

sys.path.insert(0, "/opt/trn_rl_repo")

from contextlib import ExitStack

import numpy as np

import concourse.bass as bass
import concourse.mybir as mybir
from concourse.bass_utils import run_bass_kernel_spmd
from concourse.masks import make_identity

N = 1024
P = 128            # samples per core
XC, YC, HID, S = 512, 256, 512, 64
CH = 32            # channel chunk per streamed DMA (1 MiB)
NBUF = 16          # stream buffer ring
NXV = 8            # pooled-vector ring
NF = 4             # fold buffer ring
WCOLS = 3200       # wpack padded to 12800 B/partition (512 B aligned)
F32 = mybir.dt.float32
AX = mybir.AxisListType
ALU = mybir.AluOpType
ACTF = mybir.ActivationFunctionType

# chunk table: 16 x-chunks then 8 y-chunks, 32 channels each.
NX = 16
NCHUNK = 24
# chunks folded 64->32 spatially by GPSIMD (alternating; never the first
# or the last chunks, which are split at the DMA level instead)
FOLD = [1, 3, 5, 7, 9, 11, 13, 15, 17, 19]
FOLD_RANK = {i: r for r, i in enumerate(FOLD)}

# per-transfer DMA table: (chunk, ch_lo, ch_hi)
DMAS = [(0, 0, 16), (0, 16, 32)]
DMAS += [(i, 0, CH) for i in range(1, 23)]
DMAS += [(23, q * 8, (q + 1) * 8) for q in range(4)]

# stat columns
ACOL = {16 + k: k for k in range(7)}         # A partials, chunks 16-22
SCOL = {16 + k: 7 + k for k in range(7)}     # S2 partials, chunks 16-22
ACOL[23] = 19
SCOL[23] = 20
BCOL = (14, 15)
MUCOL = (16, 17)
EYCOL = (18, 21)
NSTAT = 22

_CACHE = {}


def build_nc(debug=False):
    nc = bass.Bass()
    x = nc.dram_tensor("x", [P, XC, S], F32, kind="ExternalInput")
    y = nc.dram_tensor("y", [P, YC, S], F32, kind="ExternalInput")
    # all weights packed host-side into final SBUF layout:
    # [w1 (4k x 512h) | w2 (4k x 256c) | b1 (4) | b2 (2) | pad] per partition
    wpack = nc.dram_tensor("wpack", [P, WCOLS], F32, kind="ExternalInput")
    out_stat = nc.dram_tensor("stat", [P, NSTAT], F32, kind="ExternalOutput")
    if debug:
        dbg_muT = nc.dram_tensor("dbg_muT", [P, 2, P], F32, kind="ExternalOutput")
        dbg_mu64 = nc.dram_tensor("dbg_mu64", [P, 2 * P], F32, kind="ExternalOutput")
        dbg_xvT = nc.dram_tensor("dbg_xvT", [P, 4, P], F32, kind="ExternalOutput")
        dbg_dt = nc.dram_tensor("dbg_dt", [P, 4, CH], F32, kind="ExternalOutput")

    ctx = ExitStack()
    with ctx:
        sb = lambda name, shape: ctx.enter_context(nc.sbuf_tensor(name, shape, F32))
        ps = lambda name, shape: ctx.enter_context(nc.psum_tensor(name, shape, F32))
        sem = lambda name: ctx.enter_context(nc.semaphore(name))

        xbuf = [sb(f"xbuf{i}", [P, CH, S]) for i in range(NBUF)]
        fbuf = [sb(f"fbuf{i}", [P, CH, S // 2]) for i in range(NF)]
        xvt = sb("xvt", [P, NXV * CH])     # pooled-vector ring, contiguous

        def xvs(i, lo=0, hi=CH):           # chunk i's slot columns
            s = (i % NXV) * CH
            return xvt[:, s + lo:s + hi]
        xvT = sb("xvT", [P, 4, P])
        hT = sb("hT", [P, 4, P])
        muT = sb("muT", [P, 2, P])
        mu64 = sb("mu64", [P, 2 * P])      # 64 * mu, sample-major
        dt = sb("dt", [P, 4, CH])          # dtmp ring for y chunks
        dump = sb("dump", [P, P])          # activation main-out scratch
        stat = sb("stat_sb", [P, NSTAT])
        wsb = sb("wsb", [P, WCOLS])
        ident = sb("ident", [P, P])
        ones = sb("ones", [P, 1])

        pt = [ps(f"pt{i}", [CH, P]) for i in range(2)]
        ph = ps("ph", [P, 4, P])
        pmu = ps("pmu", [P, 2, P])
        pmu_t = ps("pmu_t", [P, 2, P])
        pey = [ps("pey0", [P, 1]), ps("pey1", [P, 1])]

        dsem = {}
        for (i, lo, hi) in DMAS:
            dsem[(i, lo)] = sem(f"d{i}_{lo}")
        dw = sem("dw")
        dout = sem("dout")
        s_const = sem("s_const")
        s_pool = sem("s_pool")
        s_fold = sem("s_fold")
        s_tp = sem("s_tp")
        s_cp = sem("s_cp")
        s_hmm = sem("s_hmm")
        s_relu = sem("s_relu")
        s_mumm = sem("s_mumm")
        s_mubias = sem("s_mubias")
        s_mutp = sem("s_mutp")
        s_mu64 = sem("s_mu64")
        s_mustat = sem("s_mustat")
        s_sub = sem("s_sub")
        s_sqa = sem("s_sqa")
        s_eymm = sem("s_eymm")
        s_ey1 = sem("s_ey1")

        def chunk_src(i, lo, hi):
            if i < NX:
                return x[:, i * CH + lo:i * CH + hi, :]
            c0 = (i - NX) * CH
            return y[:, c0 + lo:c0 + hi, :]

        def yhalf(i):       # half (0/1) and partition offset of y chunk i
            c0 = (i - NX) * CH
            return c0 // P, c0 % P

        with nc.Block() as block:

            @block.sync
            def _(e):
                for t, (i, lo, hi) in enumerate(DMAS):
                    if t == 5:
                        e.dma_start(out=wsb[:, :], in_=wpack[:, :]).then_inc(
                            dw, 16
                        )
                    if i >= NBUF and lo == 0:
                        # ring reuse guard: fold chunks free their buffer at
                        # the gpsimd fold, direct chunks at the DVE reduce
                        j = i - NBUF
                        if j in FOLD_RANK:
                            e.wait_ge(s_fold, FOLD_RANK[j] + 1)
                        else:
                            e.wait_ge(s_pool, j + 1)
                    e.dma_start(
                        out=xbuf[i % NBUF][:, lo:hi, :], in_=chunk_src(i, lo, hi)
                    ).then_inc(dsem[(i, lo)], 16)
                if debug:
                    e.wait_ge(s_mu64, 2)
                    e.dma_start(out=dbg_muT[:, :, :], in_=muT[:, :, :]).then_inc(dout, 16)
                    e.dma_start(out=dbg_mu64[:, :], in_=mu64[:, :]).then_inc(dout, 16)
                    e.dma_start(out=dbg_xvT[:, :, :], in_=xvT[:, :, :]).then_inc(dout, 16)
                    e.wait_ge(s_sub, 8)
                    e.dma_start(out=dbg_dt[:, :, :], in_=dt[:, :, :]).then_inc(dout, 16)
                e.wait_ge(dout, 32 + (64 if debug else 0))

            @block.gpsimd
            def _(e):
                make_identity(nc, ident[:, :])
                e.memset(ones[:, :], 1.0).then_inc(s_const, 1)
                # spatial fold 64->32 for the FOLD chunks (halves DVE work)
                for r, i in enumerate(FOLD):
                    e.wait_ge(dsem[(i, 0)], 16)
                    if r >= NF:
                        # fbuf ring: the DVE reduce of fold r-NF must be done
                        e.wait_ge(s_pool, FOLD[r - NF] + 1)
                    e.tensor_add(
                        fbuf[r % NF][:, :, :],
                        xbuf[i % NBUF][:, :, 0:S // 2],
                        xbuf[i % NBUF][:, :, S // 2:S],
                    ).then_inc(s_fold, 1)

            @block.vector
            def _(e):
                def pool(i):
                    if i >= NXV:
                        e.wait_ge(s_tp, i - NXV + 1)   # xv slot reuse
                    if i == 0:
                        for (lo, hi) in ((0, 16), (16, 32)):
                            e.wait_ge(dsem[(0, lo)], 16)
                            inst = e.tensor_reduce(
                                xvs(0, lo, hi),
                                xbuf[0][:, lo:hi, :],
                                axis=AX.X, op=ALU.add,
                            )
                    elif i == 23:
                        for q in range(4):
                            e.wait_ge(dsem[(23, q * 8)], 16)
                            inst = e.tensor_reduce(
                                xvs(23, q * 8, (q + 1) * 8),
                                xbuf[7][:, q * 8:(q + 1) * 8, :],
                                axis=AX.X, op=ALU.add,
                            )
                    elif i in FOLD_RANK:
                        r = FOLD_RANK[i]
                        e.wait_ge(s_fold, r + 1)
                        inst = e.tensor_reduce(
                            xvs(i), fbuf[r % NF][:, :, :],
                            axis=AX.X, op=ALU.add,
                        )
                    else:
                        e.wait_ge(dsem[(i, 0)], 16)
                        inst = e.tensor_reduce(
                            xvs(i), xbuf[i % NBUF][:, :, :],
                            axis=AX.X, op=ALU.add,
                        )
                    inst.then_inc(s_pool, 1)

                def sub(i):
                    k = i - NX
                    if k >= 4:
                        e.wait_ge(s_sqa, k - 3)        # dt ring reuse
                    m, q0 = yhalf(i)
                    e.tensor_sub(
                        dt[:, k % 4, :], xvs(i),
                        mu64[:, m * P + q0:m * P + q0 + CH],
                    ).then_inc(s_sub, 1)

                # NOTE: a sub(i) must never directly follow pool(i): a DVE
                # op reading the slot a reduce just wrote loses the last 16
                # bytes x 8 partitions (same-engine RAW hazard, HW-observed).
                # Subs therefore trail the pools by one chunk.
                for i in range(18):
                    pool(i)
                    if i == 17:
                        # mu stats + first sub (mu ready ~x-stream end)
                        e.wait_ge(s_mubias, 2)
                        e.tensor_reduce(
                            stat[:, MUCOL[0]:MUCOL[0] + 1], muT[:, 0, :],
                            axis=AX.X, op=ALU.add,
                        )
                        e.tensor_reduce(
                            stat[:, MUCOL[1]:MUCOL[1] + 1], muT[:, 1, :],
                            axis=AX.X, op=ALU.add,
                        ).then_inc(s_mustat, 1)
                        e.wait_ge(s_mu64, 2)
                        sub(16)
                for i in range(18, 24):
                    pool(i)
                    sub(i - 1)
                # Ey half0: its matmuls only involve chunks 16-19, done long
                # ago; the copy doubles as a pool(23)->sub(23) separator
                e.wait_ge(s_eymm, 2)
                e.tensor_scalar_mul(
                    stat[:, EYCOL[0]:EYCOL[0] + 1], pey[0][:, :], 1.0
                )
                sub(23)
                e.wait_ge(s_eymm, 4)
                e.tensor_scalar_mul(
                    stat[:, EYCOL[1]:EYCOL[1] + 1], pey[1][:, :], 1.0
                ).then_inc(s_ey1, 1)

            @block.tensor
            def _(e):
                e.wait_ge(s_const, 1)
                for i in range(NX):
                    e.wait_ge(s_pool, i + 1)
                    if i >= 2:
                        e.wait_ge(s_cp, i - 1)
                    e.transpose(
                        pt[i % 2][:, :], xvs(i), ident[:, :]
                    ).then_inc(s_tp, 1)
                # h = x_vec @ W1: fp32 accumulation groups must stay
                # contiguous (interleaving groups miscompiles)
                e.wait_ge(s_cp, NX)
                e.wait_ge(dw, 16)
                for m in range(4):
                    for k in range(4):
                        mm = e.matmul(
                            ph[:, m, :],
                            wsb[:, k * 512 + m * P:k * 512 + (m + 1) * P],
                            xvT[:, k, :],
                            start=(k == 0),
                            stop=(k == 3),
                        )
                mm.then_inc(s_hmm, 1)
                e.wait_ge(s_relu, 4)
                for m in range(2):
                    for k in range(4):
                        mm = e.matmul(
                            pmu[:, m, :],
                            wsb[:, 2048 + k * 256 + m * P:
                                2048 + k * 256 + (m + 1) * P],
                            hT[:, k, :],
                            start=(k == 0),
                            stop=(k == 3),
                        )
                mm.then_inc(s_mumm, 1)
                # mu back to sample-major for the y epilogue
                e.wait_ge(s_mubias, 2)
                for m in range(2):
                    e.transpose(
                        pmu_t[:, m, :], muT[:, m, :], ident[:, :]
                    ).then_inc(s_mutp, 1)
                # Ey partition sums: ones-matmul per y chunk PAIR (the two
                # chunks are adjacent xvt slots -> one contiguous lhsT, and
                # the output base partition stays in {0, 64}).
                # Gate on mu64: a matmul into the pey bank corrupts ACT's
                # concurrent pmu_t reads.
                e.wait_ge(s_mu64, 2)
                for k in range(4):
                    i = NX + 2 * k + 1          # later chunk of the pair
                    e.wait_ge(s_pool, i + 1)
                    e.matmul(
                        pey[k // 2][(k % 2) * 64:(k % 2) * 64 + 64, :],
                        xvt[:, 64 * k:64 * (k + 1)],
                        ones[:, :], start=True, stop=True,
                    ).then_inc(s_eymm, 1)

            @block.scalar
            def _(e):
                for i in range(NX):
                    e.wait_ge(s_tp, i + 1)
                    # fold the 1/64 spatial mean into the transpose copy
                    c0 = i * CH
                    e.activation(
                        xvT[c0 % P:c0 % P + CH, c0 // P, :], pt[i % 2][:, :],
                        ACTF.Copy, scale=1.0 / S,
                    ).then_inc(s_cp, 1)
                e.wait_ge(s_hmm, 1)
                for m in range(4):
                    e.activation(
                        hT[:, m, :], ph[:, m, :], ACTF.Relu,
                        bias=wsb[:, 3072 + m:3073 + m],
                    ).then_inc(s_relu, 1)
                e.wait_ge(s_mumm, 1)
                for m in range(2):
                    e.activation(
                        muT[:, m, :], pmu[:, m, :], ACTF.Identity,
                        bias=wsb[:, 3076 + m:3077 + m],
                    ).then_inc(s_mubias, 1)
                for m in range(2):
                    e.wait_ge(s_mutp, m + 1)
                    e.activation(
                        mu64[:, m * P:(m + 1) * P], pmu_t[:, m, :],
                        ACTF.Copy, scale=float(S),
                    ).then_inc(s_mu64, 1)
                # B = sum_i mu_i[d]^2 per channel (true mu scale)
                for m in range(2):
                    e.activation(
                        dump[:, :], muT[:, m, :], ACTF.Square,
                        accum_out=stat[:, BCOL[m]:BCOL[m] + 1],
                    )
                for i in range(NX, NCHUNK):
                    k = i - NX
                    e.wait_ge(s_pool, i + 1)
                    e.activation(
                        dump[:, 0:CH], xvs(i), ACTF.Square,
                        accum_out=stat[:, SCOL[i]:SCOL[i] + 1],
                    )
                    e.wait_ge(s_sub, k + 1)
                    e.activation(
                        dump[:, 0:CH], dt[:, k % 4, :], ACTF.Square,
                        accum_out=stat[:, ACOL[i]:ACOL[i] + 1],
                    ).then_inc(s_sqa, 1)
                    if i == 22:
                        # early ship: everything the last chunk doesn't touch
                        e.wait_ge(s_mustat, 1)
                        e.dma_start(
                            out=out_stat[:, 0:18], in_=stat[:, 0:18]
                        ).then_inc(dout, 16)
                e.wait_ge(s_ey1, 1)
                e.dma_start(
                    out=out_stat[:, 18:NSTAT], in_=stat[:, 18:NSTAT]
                ).then_inc(dout, 16)

    return nc


def _get_nc():
    if "nc" not in _CACHE:
        _CACHE["nc"] = build_nc()
    return _CACHE["nc"]


def make_in_maps(x_samples, y_samples, W1, b1, W2, b2):
    xs = np.ascontiguousarray(
        np.asarray(x_samples, np.float32).reshape(N, XC, S)
    )
    ys = np.ascontiguousarray(
        np.asarray(y_samples, np.float32).reshape(N, YC, S)
    )
    wp = np.zeros((P, WCOLS), np.float32)
    wp[:, :2048] = (
        np.asarray(W1, np.float32).reshape(4, P, HID).transpose(1, 0, 2).reshape(P, 2048)
    )
    wp[:, 2048:3072] = (
        np.asarray(W2, np.float32).reshape(4, P, YC).transpose(1, 0, 2).reshape(P, 1024)
    )
    wp[:, 3072:3076] = np.asarray(b1, np.float32).reshape(4, P).T
    wp[:, 3076:3078] = np.asarray(b2, np.float32).reshape(2, P).T
    wp = np.ascontiguousarray(wp)
    in_maps = []
    for c in range(8):
        in_maps.append(
            {
                "x": np.ascontiguousarray(xs[c * P:(c + 1) * P]),
                "y": np.ascontiguousarray(ys[c * P:(c + 1) * P]),
                "wpack": wp,
            }
        )
    return in_maps


def combine(results):
    A = B = S2 = 0.0
    EyN = np.zeros(YC, np.float64)
    MuN = np.zeros(YC, np.float64)
    for c in range(8):
        st = results[c]["stat"].astype(np.float64)       # (128, 22)
        A += st[:, 0:7].sum() + st[:, 19].sum()
        S2 += st[:, 7:14].sum() + st[:, 20].sum()
        B += st[:, 14:16].sum()
        MuN += np.concatenate([st[:, 16], st[:, 17]])
        EyN += np.concatenate([st[:, 18], st[:, 21]])
    A /= 4096.0
    S2 /= 4096.0
    ey = EyN / 64.0 / N
    mu = MuN / N
    loss = -(A / N) / 2.0 + 0.5 * (S2 / N - 2.0 * float(mu @ ey) + B / N)
    return np.float32(loss)


def run(inputs, **kwargs):
    nc = _get_nc()
    in_maps = make_in_maps(**inputs)
    res = run_bass_kernel_spmd(nc, in_maps, core_ids=list(range(8)), **kwargs)
    return combine(res.results), res


def kernel(x_samples, y_samples, W1, b1, W2, b2):
    loss, _ = run(
        dict(
            x_samples=x_samples,
            y_samples=y_samples,
            W1=W1,
            b1=b1,
            W2=W2,
            b2=b2,
        )
    )
    return loss
